# revision 6
# baseline (speedup 1.0000x reference)
"""LSAGEDirected GNN message-passing kernel for 8 Trainium2 NeuronCores.

Algorithm (matches the jax reference):
  3 directed mean-aggregation hops (dim 64 -> 128 -> 256 -> 512), then a
  512x128 linear projection.

Mapping:
  - Nodes sharded round-robin across the 8 cores (owner = v % 8); each hop's
    feature table is replicated per core (pair-shared) via AllGather.
  - Per (core, direction): destination nodes grouped into 128-node blocks
    (lexsorted by per-parity gather counts), edges assigned to slot columns,
    padded with a zero-row reference.
  - Gathers run on gpsimd dma_gather (int16 indices); the 50k-row table is
    addressed through even/odd row views (stride = 2 rows) so idx = row >> 1
    fits int16.  Slot columns are parity-split per block.
  - Per block: VectorE segmented reduce (fp32) + inv-degree scale -> bf16
    staging; one batched dma_scatter_add per direction writes the local
    [stride_loc, 2d] buffer (zero-initialized); AllGather replicates.
  - Final hop writes a local [stride_loc, 512] bf16 buffer; a matmul pass
    (TensorE transpose + 4-chunk accumulation against W^T) produces the
    [6272, 128] fp32 output rows per core; the host inverse-permutes.
"""

import sys
import numpy as np

sys.path.insert(0, "/opt/trn_rl_repo")

import ml_dtypes  # noqa: E402
import concourse.bass as bass  # noqa: E402
import concourse.bacc as bacc  # noqa: E402
import concourse.tile as tile  # noqa: E402
from concourse import mybir  # noqa: E402
from concourse import bass_utils  # noqa: E402
from concourse.masks import make_identity  # noqa: E402

BF16 = mybir.dt.bfloat16
F32 = mybir.dt.float32
I16 = mybir.dt.int16

N_CORES = 8
MAX_IDX_PER_GATHER = 5120   # 40 slot columns per parity per chunk
BF = ml_dtypes.bfloat16


# ----------------------------------------------------------------------------
# host-side planning
# ----------------------------------------------------------------------------

def _wrap_idx16(flat):
    """dma_gather/scatter idx layout: j -> [j%16, j//16], replicated x8."""
    assert flat.size % 16 == 0
    w = flat.reshape(-1, 16).T.astype(np.int16)  # [16, n/16]
    return np.tile(w, (8, 1))                    # [128, n/16]


def _plan_direction(keys_per_core, gidx_per_core, n_loc, npad, stride_loc):
    """Plan one direction (shared block structure across cores).

    keys_per_core[p]: local dst-side node id per edge (0..n_loc-1)
    gidx_per_core[p]: global table row of the gathered endpoint per edge
    Returns dict with uniform structure + per-core arrays.
    """
    nblk = npad // 128
    percore = []
    for p in range(N_CORES):
        k = keys_per_core[p]
        g = gidx_per_core[p]
        par = (g & 1).astype(np.int64)
        lo = np.bincount(k * 2 + par, minlength=2 * n_loc).reshape(n_loc, 2)
        deg = lo.sum(1)
        order = np.lexsort((lo[:, 1], lo[:, 0]))  # by (lo, then hi)
        percore.append((k, g, par, lo, deg, order))

    # uniform per-block parity slot counts = max over cores of sorted profiles
    S = np.zeros((nblk, 2), np.int64)
    for (k, g, par, lo, deg, order) in percore:
        L = lo[order]
        L = np.vstack([L, np.zeros((npad - n_loc, 2), np.int64)])
        m = L.reshape(nblk, 128, 2).max(axis=1)
        S = np.maximum(S, m)

    # chunks: consecutive blocks, per-parity columns <= MAX/128
    cap = MAX_IDX_PER_GATHER // 128
    chunks = []  # list of (b0, b1)
    b0 = 0
    while b0 < nblk:
        b1 = b0 + 1
        while (b1 < nblk
               and S[b0:b1 + 1, 0].sum() <= cap
               and S[b0:b1 + 1, 1].sum() <= cap):
            b1 += 1
        chunks.append((b0, b1))
        b0 = b1

    total_cols = int(S.sum())
    # per-core idx arrays + inv + scatter rows
    zrow = stride_loc - 128  # local row (in core 0's stripe) used as zero row
    # zero row must exist with both parities: use rows zrow (even g?) handled
    # by caller: g_zero_even / g_zero_odd passed in gidx domain:
    g_zero = [zrow * 1, zrow + 1]  # rows zrow, zrow+1 of core0 stripe == global rows
    idx16 = np.zeros((N_CORES, 128, (total_cols * 128) // 16), np.int16)
    inv = np.zeros((N_CORES, 128, nblk), np.float32)
    rows = np.zeros((N_CORES, 128, nblk), np.int32)

    for p in range(N_CORES):
        k, g, par, lo, deg, order = percore[p]
        rank_of = np.empty(n_loc, np.int64)
        rank_of[order] = np.arange(n_loc)
        er = rank_of[k]                       # block-rank of each edge's node
        eb = er // 128                         # block id
        ei = er % 128                          # partition within block
        # slot within (node, parity) run
        so = np.lexsort((g, par, er))          # group by node, evens first
        er_s, par_s, g_s = er[so], par[so], g[so]
        # run starts
        key2 = er_s * 2 + par_s
        cnt = np.bincount(key2, minlength=2 * n_loc)
        starts = np.zeros(2 * n_loc + 1, np.int64)
        np.cumsum(cnt, out=starts[1:])
        within = np.arange(len(so)) - starts[key2]
        # flat destination: per chunk layout [evens of blocks][odds of blocks]
        # compute global column of each edge
        colbase = np.zeros((nblk, 2), np.int64)
        c = 0
        for (b0c, b1c) in chunks:
            for b in range(b0c, b1c):
                colbase[b, 0] = c
                c += S[b, 0]
            for b in range(b0c, b1c):
                colbase[b, 1] = c
                c += S[b, 1]
        assert c == total_cols
        ecol = colbase[er_s // 128, par_s] + within
        flatpos = ecol * 128 + (er_s % 128)
        flat = np.empty(total_cols * 128, np.int64)
        # padding: zero rows (parity matching the column's parity region)
        # build per-column parity map
        colpar = np.zeros(total_cols, np.int64)
        for (b0c, b1c) in chunks:
            for b in range(b0c, b1c):
                colpar[colbase[b, 1]:colbase[b, 1] + S[b, 1]] = 1
        flat[:] = np.repeat(np.where(colpar == 0, g_zero[0], g_zero[1]), 128) >> 1
        flat[flatpos] = g_s >> 1
        idx16[p] = _wrap_idx16(flat)

        invv = np.zeros(npad, np.float32)
        invv[:n_loc] = 1.0 / np.maximum(deg[order], 1.0)
        # nodes with deg 0 produce sum 0 anyway; dummies -> 0
        inv[p] = invv.reshape(nblk, 128).T

        # scatter target rows (canonical local row = node_local // 1 ... pos)
        rt = np.empty(npad, np.int64)
        rt[:n_loc] = order          # canonical local row of node = its local id
        rt[n_loc:] = stride_loc - 126 + np.arange(npad - n_loc)  # junk rows
        rows[p] = rt.reshape(nblk, 128).T

    # block ranges within each chunk's tile
    chunk_meta = []
    for (b0c, b1c) in chunks:
        ncols_lo = int(S[b0c:b1c, 0].sum())
        ncols_hi = int(S[b0c:b1c, 1].sum())
        blocks = []
        ce = 0
        co = ncols_lo
        for b in range(b0c, b1c):
            blocks.append((b, ce, ce + int(S[b, 0]), co, co + int(S[b, 1])))
            ce += int(S[b, 0])
            co += int(S[b, 1])
        chunk_meta.append(dict(b0=b0c, b1=b1c, ncols_lo=ncols_lo,
                               ncols_hi=ncols_hi, blocks=blocks))
    return dict(chunks=chunk_meta, total_cols=total_cols, idx16=idx16,
                inv=inv, rows=rows,
                order=[pc[5] for pc in percore])


def _make_plan(feature, edge_index, n_nodes, n_loc, npad, stride_loc,
               in_dim, out_dim):
    src = np.asarray(edge_index[0], np.int64)
    dst = np.asarray(edge_index[1], np.int64)
    owner = lambda v: v % N_CORES
    pos = lambda v: v // N_CORES
    grow = lambda v: (v % N_CORES) * stride_loc + v // N_CORES

    g_src = grow(src)
    g_dst = grow(dst)

    keys_in, gidx_in, keys_out, gidx_out = [], [], [], []
    for p in range(N_CORES):
        m = (dst % N_CORES) == p
        keys_in.append(pos(dst[m]))
        gidx_in.append(g_src[m])
        m = (src % N_CORES) == p
        keys_out.append(pos(src[m]))
        gidx_out.append(g_dst[m])

    plan_in = _plan_direction(keys_in, gidx_in, n_loc, npad, stride_loc)
    plan_out = _plan_direction(keys_out, gidx_out, n_loc, npad, stride_loc)

    # canonical feature table T0 (fp32), zero rows included
    ntab = N_CORES * stride_loc
    T0 = np.zeros((ntab, in_dim), np.float32)
    v = np.arange(n_nodes)
    T0[grow(v)] = np.asarray(feature, np.float32)

    # scatter row idx arrays (int16, wrapped) per direction
    def rows16(plan):
        out = np.zeros((N_CORES, 128, npad // 16), np.int16)
        for p in range(N_CORES):
            r = plan["rows"][p]            # [128, nblk]
            flat = r.T.reshape(-1)         # block-major, partition within
            out[p] = _wrap_idx16(flat.astype(np.int64))
        return out

    return dict(p_in=plan_in, p_out=plan_out, T0=T0,
                rows16_in=rows16(plan_in), rows16_out=rows16(plan_out))


# ----------------------------------------------------------------------------
# device program
# ----------------------------------------------------------------------------

def _build_program(plan, n_loc, npad, stride_loc, in_dim, out_dim):
    nblk = npad // 128
    ntab = N_CORES * stride_loc
    d1, d2, d3 = in_dim, in_dim * 2, in_dim * 4   # table dims per hop input
    dims = [d1, d2, d3]
    fin = in_dim * 8

    nc = bacc.Bacc("TRN2", target_bir_lowering=False, debug=False,
                   num_devices=N_CORES, num_swdge_queues=4)

    t0 = nc.dram_tensor("t0", [ntab, d1], F32, kind="ExternalInput")
    ci = plan["p_in"]["total_cols"] * 8
    co = plan["p_out"]["total_cols"] * 8
    idx_in = nc.dram_tensor("idx_in", [128, ci], I16, kind="ExternalInput")
    idx_out = nc.dram_tensor("idx_out", [128, co], I16, kind="ExternalInput")
    inv_in = nc.dram_tensor("inv_in", [128, nblk], F32, kind="ExternalInput")
    inv_out = nc.dram_tensor("inv_out", [128, nblk], F32, kind="ExternalInput")
    r16_in = nc.dram_tensor("r16_in", [128, npad // 16], I16, kind="ExternalInput")
    r16_out = nc.dram_tensor("r16_out", [128, npad // 16], I16, kind="ExternalInput")
    wt_d = nc.dram_tensor("wt", [128, (fin // 128) * out_dim], BF16, kind="ExternalInput")
    bias_d = nc.dram_tensor("bias", [128, out_dim], F32, kind="ExternalInput")
    out_d = nc.dram_tensor("out", [npad, out_dim], F32, kind="ExternalOutput")

    # hop tables + local buffers
    L1 = nc.dram_tensor("L1", [stride_loc, 2 * d1], BF16, kind="Internal")
    L2 = nc.dram_tensor("L2", [stride_loc, 2 * d2], BF16, kind="Internal")
    L3 = nc.dram_tensor("L3", [stride_loc, 2 * d3], BF16, kind="Internal")
    T1 = nc.dram_tensor("T1", [ntab, d2], BF16, kind="Internal", addr_space="Shared")
    T2 = nc.dram_tensor("T2", [ntab, d3], BF16, kind="Internal", addr_space="Shared")
    groups = [list(range(N_CORES))]

    qn = [0]

    def next_q():
        qn[0] = (qn[0] + 1) % 4
        return qn[0]

    with tile.TileContext(nc) as tc:
        with tc.tile_pool(name="const", bufs=1) as cpool, \
             tc.tile_pool(name="gath", bufs=2) as gpool, \
             tc.tile_pool(name="accp", bufs=4) as apool, \
             tc.tile_pool(name="stag", bufs=2) as spool, \
             tc.tile_pool(name="mm", bufs=3) as mpool, \
             tc.tile_pool(name="ps", bufs=2, space="PSUM") as pspool, \
             tc.tile_pool(name="ps2", bufs=2, space="PSUM") as ps2pool:

            idx_in_t = cpool.tile([128, ci], I16, tag="idxin")
            nc.sync.dma_start(idx_in_t[:], idx_in[:])
            idx_out_t = cpool.tile([128, co], I16, tag="idxout")
            nc.sync.dma_start(idx_out_t[:], idx_out[:])
            inv_in_t = cpool.tile([128, nblk], F32, tag="invin")
            nc.sync.dma_start(inv_in_t[:], inv_in[:])
            inv_out_t = cpool.tile([128, nblk], F32, tag="invout")
            nc.sync.dma_start(inv_out_t[:], inv_out[:])
            r16_in_t = cpool.tile([128, npad // 16], I16, tag="r16in")
            nc.sync.dma_start(r16_in_t[:], r16_in[:])
            r16_out_t = cpool.tile([128, npad // 16], I16, tag="r16out")
            nc.sync.dma_start(r16_out_t[:], r16_out[:])
            zero_t = cpool.tile([128, 2048], BF16, tag="zt")
            nc.vector.memset(zero_t[:], 0.0)

            def zero_dram(buf, rows_, width):
                tot = rows_ * width          # bf16 elements
                assert tot % 128 == 0
                per = tot // 128
                view = buf[:].rearrange("a b -> (a b)").rearrange(
                    "(p n) -> p n", p=128)
                off = 0
                while off < per:
                    n = min(2048, per - off)
                    nc.sync.dma_start(view[:, off:off + n], zero_t[:, :n])
                    off += n

            def run_side(plan_dir, idx_t, inv_t, r16_t, table_view_lo,
                         table_view_hi, d, dst_buf, half, hopname):
                """One direction of one hop."""
                dd = d  # gathered feature width
                icol = [0]  # consumed idx columns (in idx16 col units = /16)
                stag_written = 0
                stg = spool.tile([128, nblk, dd], BF16, tag=f"st")
                for ch in plan_dir["chunks"]:
                    ncl, nch_ = ch["ncols_lo"], ch["ncols_hi"]
                    if ncl + nch_ == 0:
                        tile_g = None
                    else:
                        tile_g = gpool.tile([128, ncl + nch_, dd], BF16,
                                            tag="gtile")
                    for (par, ncols, view) in ((0, ncl, table_view_lo),
                                               (1, nch_, table_view_hi)):
                        if ncols == 0:
                            continue
                        nidx = 128 * ncols
                        seg = idx_t[:, icol[0]:icol[0] + nidx // 16]
                        o0 = 0 if par == 0 else ncl
                        nc.gpsimd.dma_gather(
                            out_ap=tile_g[:, o0:o0 + ncols, :],
                            in_ap=view,
                            idxs_ap=seg,
                            num_idxs=nidx,
                            num_idxs_reg=nidx,
                            elem_size=dd,
                            elem_step=2 * dd,
                            single_packet=False,
                            queue_num=next_q(),
                        )
                        icol[0] += nidx // 16
                    for (b, e0, e1, o0, o1) in ch["blocks"]:
                        acc = apool.tile([128, dd], F32, tag="acc")
                        ne, no = e1 - e0, o1 - o0
                        if ne > 0:
                            nc.vector.tensor_reduce(
                                acc[:], tile_g[:, e0:e1, :].rearrange(
                                    "p s d -> p d s"),
                                axis=mybir.AxisListType.X,
                                op=mybir.AluOpType.add)
                            if no > 0:
                                acc2 = apool.tile([128, dd], F32, tag="acc2")
                                nc.vector.tensor_reduce(
                                    acc2[:], tile_g[:, o0:o1, :].rearrange(
                                        "p s d -> p d s"),
                                    axis=mybir.AxisListType.X,
                                    op=mybir.AluOpType.add)
                                nc.vector.tensor_tensor(
                                    out=acc[:], in0=acc[:], in1=acc2[:],
                                    op=mybir.AluOpType.add)
                        elif no > 0:
                            nc.vector.tensor_reduce(
                                acc[:], tile_g[:, o0:o1, :].rearrange(
                                    "p s d -> p d s"),
                                axis=mybir.AxisListType.X,
                                op=mybir.AluOpType.add)
                        else:
                            nc.vector.memset(acc[:], 0.0)
                        nc.vector.tensor_scalar(
                            out=stg[:, b, :], in0=acc[:],
                            scalar1=inv_t[:, b:b + 1], scalar2=None,
                            op0=mybir.AluOpType.mult)
                        stag_written += 1
                # batched scatter-add into dst_buf column half
                w2 = dst_buf.shape[1]
                hv = dst_buf[:].rearrange("n (h d) -> n h d", h=2)[:, half, :]
                nidx = nblk * 128
                nc.gpsimd.dma_scatter_add(
                    out_ap=hv,
                    in_ap=stg[:],
                    idxs_ap=r16_t[:],
                    num_idxs=nidx,
                    num_idxs_reg=nidx,
                    elem_size=dd,
                    elem_step=w2,
                    single_packet=False,
                    queue_num=next_q(),
                )

            # ---- hop 1 (gathers fp32 from t0) ----
            # NOTE: gather tiles for hop1 are fp32; allocate separately
            def run_side_f32(plan_dir, idx_t, inv_t, r16_t, tlo, thi, d,
                             dst_buf, half):
                icol = [0]
                stg = spool.tile([128, nblk, d], BF16, tag="st")
                for ch in plan_dir["chunks"]:
                    ncl, nch_ = ch["ncols_lo"], ch["ncols_hi"]
                    tile_g = None
                    if ncl + nch_ > 0:
                        tile_g = gpool.tile([128, ncl + nch_, d], F32,
                                            tag="gtile")
                    for (par, ncols, view) in ((0, ncl, tlo), (1, nch_, thi)):
                        if ncols == 0:
                            continue
                        nidx = 128 * ncols
                        seg = idx_t[:, icol[0]:icol[0] + nidx // 16]
                        o0 = 0 if par == 0 else ncl
                        nc.gpsimd.dma_gather(
                            out_ap=tile_g[:, o0:o0 + ncols, :],
                            in_ap=view, idxs_ap=seg,
                            num_idxs=nidx, num_idxs_reg=nidx,
                            elem_size=d, elem_step=2 * d,
                            single_packet=False, queue_num=next_q())
                        icol[0] += nidx // 16
                    for (b, e0, e1, o0, o1) in ch["blocks"]:
                        acc = apool.tile([128, d], F32, tag="acc")
                        ne, no = e1 - e0, o1 - o0
                        if ne > 0:
                            nc.vector.tensor_reduce(
                                acc[:], tile_g[:, e0:e1, :].rearrange(
                                    "p s d -> p d s"),
                                axis=mybir.AxisListType.X,
                                op=mybir.AluOpType.add)
                            if no > 0:
                                acc2 = apool.tile([128, d], F32, tag="acc2")
                                nc.vector.tensor_reduce(
                                    acc2[:], tile_g[:, o0:o1, :].rearrange(
                                        "p s d -> p d s"),
                                    axis=mybir.AxisListType.X,
                                    op=mybir.AluOpType.add)
                                nc.vector.tensor_tensor(
                                    out=acc[:], in0=acc[:], in1=acc2[:],
                                    op=mybir.AluOpType.add)
                        elif no > 0:
                            nc.vector.tensor_reduce(
                                acc[:], tile_g[:, o0:o1, :].rearrange(
                                    "p s d -> p d s"),
                                axis=mybir.AxisListType.X,
                                op=mybir.AluOpType.add)
                        else:
                            nc.vector.memset(acc[:], 0.0)
                        nc.vector.tensor_scalar(
                            out=stg[:, b, :], in0=acc[:],
                            scalar1=inv_t[:, b:b + 1], scalar2=None,
                            op0=mybir.AluOpType.mult)
                w2 = dst_buf.shape[1]
                hv = dst_buf[:].rearrange("n (h d) -> n h d", h=2)[:, half, :]
                nidx = nblk * 128
                nc.gpsimd.dma_scatter_add(
                    out_ap=hv, in_ap=stg[:], idxs_ap=r16_t[:],
                    num_idxs=nidx, num_idxs_reg=nidx,
                    elem_size=d, elem_step=w2,
                    single_packet=False, queue_num=next_q())

            def tviews(tbl, d):
                r = tbl[:].rearrange("(n two) d -> n (two d)", two=2)
                return r[:, 0:d], r[:, d:2 * d]

            # constants for the matmul stage (hoisted so Pool work is early)
            ident = cpool.tile([128, 128], BF16, tag="ident")
            make_identity(nc, ident[:])
            wt_t = cpool.tile([128, (fin // 128) * out_dim], BF16, tag="wt")
            nc.sync.dma_start(wt_t[:], wt_d[:])
            bias_t = cpool.tile([128, out_dim], F32, tag="bias")
            nc.sync.dma_start(bias_t[:], bias_d[:])

            # hop 1
            zero_dram(L1, stride_loc, 2 * d1)
            zero_dram(L2, stride_loc, 2 * d2)
            zero_dram(L3, stride_loc, 2 * d3)
            t0lo, t0hi = tviews(t0, d1)
            run_side_f32(plan["p_in"], idx_in_t, inv_in_t, r16_in_t,
                         t0lo, t0hi, d1, L1, 0)
            run_side_f32(plan["p_out"], idx_out_t, inv_out_t, r16_out_t,
                         t0lo, t0hi, d1, L1, 1)
            nc.gpsimd.collective_compute(
                "AllGather", mybir.AluOpType.bypass, replica_groups=groups,
                ins=[L1[:]], outs=[T1[:]])

            # hop 2
            t1lo, t1hi = tviews(T1, d2)
            run_side(plan["p_in"], idx_in_t, inv_in_t, r16_in_t,
                     t1lo, t1hi, d2, L2, 0, "h2")
            run_side(plan["p_out"], idx_out_t, inv_out_t, r16_out_t,
                     t1lo, t1hi, d2, L2, 1, "h2")
            nc.gpsimd.collective_compute(
                "AllGather", mybir.AluOpType.bypass, replica_groups=groups,
                ins=[L2[:]], outs=[T2[:]])

            # hop 3 (no allgather; local result only)
            t2lo, t2hi = tviews(T2, d3)
            run_side(plan["p_in"], idx_in_t, inv_in_t, r16_in_t,
                     t2lo, t2hi, d3, L3, 0, "h3")
            run_side(plan["p_out"], idx_out_t, inv_out_t, r16_out_t,
                     t2lo, t2hi, d3, L3, 1, "h3")

            # ---- final matmul: out[npad, out_dim] = L3[:npad] @ W.T + b ----
            nchk = fin // 128
            for b in range(npad // 128):
                h3 = mpool.tile([128, fin], BF16, tag="h3")
                nc.sync.dma_start(h3[:], L3[b * 128:(b + 1) * 128, :])
                h3ts = []
                for c in range(nchk):
                    ptr = pspool.tile([128, 128], BF16, tag="ptr")
                    nc.tensor.transpose(
                        ptr[:], h3[:, c * 128:(c + 1) * 128], ident[:])
                    h3t = mpool.tile([128, 128], BF16, tag=f"h3t{c}")
                    nc.vector.tensor_copy(h3t[:], ptr[:])
                    h3ts.append(h3t)
                pout = ps2pool.tile([128, out_dim], F32, tag="pout")
                for c in range(nchk):
                    nc.tensor.matmul(
                        pout[:], lhsT=h3ts[c][:],
                        rhs=wt_t[:, c * out_dim:(c + 1) * out_dim],
                        start=(c == 0), stop=(c == nchk - 1))
                ob = mpool.tile([128, out_dim], F32, tag="ob")
                nc.vector.tensor_tensor(
                    out=ob[:], in0=pout[:], in1=bias_t[:],
                    op=mybir.AluOpType.add)
                nc.sync.dma_start(out_d[b * 128:(b + 1) * 128, :], ob[:])

    # Align SWDGE queue choice with Tile's DMASW completion-sem lanes:
    # a semaphore may only be updated from one SWDGE queue, and Tile assigns
    # lanes round-robin in scheduled order, so derive queue = lane % 4.
    for fn in nc.m.functions:
        for bb in fn.blocks:
            for ins in bb.instructions:
                if type(ins).__name__ in ("InstDMAGatherAnt",
                                          "InstDMAScatterAddAnt"):
                    si = ins.sync_info
                    if si is None or not si.on_update:
                        continue
                    name = si.on_update[0].ant_name
                    if name and name.startswith("DMASW"):
                        lane = int(name[5:].split("_")[0])
                        ins.queue_num = lane % 4

    nc.compile()
    return nc


# ----------------------------------------------------------------------------
# entry point
# ----------------------------------------------------------------------------

def _run(feature, edge_index, W, b, n_nodes, in_dim, out_dim, sim=False,
         trace=False):
    n_loc = n_nodes // N_CORES
    npad = ((n_loc + 127) // 128) * 128
    stride_loc = npad + 128           # spare rows: zero rows + junk
    fin = in_dim * 8

    plan = _make_plan(feature, edge_index, n_nodes, n_loc, npad, stride_loc,
                      in_dim, out_dim)
    nc = _build_program(plan, n_loc, npad, stride_loc, in_dim, out_dim)

    WTf = np.ascontiguousarray(np.asarray(W, np.float32).T)  # [fin, out]
    # arrange as [128, nchk*out]: [p, c*out+o] = WT[c*128+p, o]
    nchk = fin // 128
    WT = WTf.reshape(nchk, 128, out_dim).transpose(1, 0, 2).reshape(
        128, nchk * out_dim).astype(BF)
    bias_rep = np.tile(np.asarray(b, np.float32)[None, :], (128, 1))

    in_maps = []
    for p in range(N_CORES):
        in_maps.append({
            "t0": plan["T0"],
            "idx_in": plan["p_in"]["idx16"][p],
            "idx_out": plan["p_out"]["idx16"][p],
            "inv_in": plan["p_in"]["inv"][p],
            "inv_out": plan["p_out"]["inv"][p],
            "r16_in": plan["rows16_in"][p],
            "r16_out": plan["rows16_out"][p],
            "wt": WT,
            "bias": bias_rep.astype(np.float32),
        })

    if sim:
        from concourse.bass_interp import MultiCoreSim

        class _R:
            pass
        msim = MultiCoreSim(nc, num_cores=N_CORES, trace=False)
        cores = list(msim.cores.values())
        for p, core in enumerate(cores):
            for k, v in in_maps[p].items():
                core.tensor(k)[:] = v
        msim.simulate(check_with_hw=False)
        res = _R()
        res.results = [{"out": np.array(core.tensor("out"))} for core in cores]
        res.exec_time_ns = None
    else:
        res = bass_utils.run_bass_kernel_spmd(
            nc, in_maps, core_ids=list(range(N_CORES)), trace=trace)

    out = np.empty((n_nodes, out_dim), np.float32)
    v = np.arange(n_nodes)
    out[v] = 0.0
    for p in range(N_CORES):
        op = res.results[p]["out"]          # [npad, out_dim], canonical rows
        nodes = np.arange(p, n_nodes, N_CORES)   # owner p, pos = v//8
        out[nodes] = op[:len(nodes)]
    return out, res


def kernel(feature, edge_index, W, b):
    out, _ = _run(np.asarray(feature), np.asarray(edge_index),
                  np.asarray(W), np.asarray(b),
                  n_nodes=50000, in_dim=64, out_dim=128)
    return out
